# revision 1
# baseline (speedup 1.0000x reference)
"""AUGRU (attention-update GRU) Trainium2 kernel.

Problem: B=1024, T=200, I=H=128, fp32 in/out.
  h_t = (1 - a_t*z_t) * h_{t-1} + a_t*z_t * tanh(xh_t + Uh (r_t*h_{t-1}))
  z_t = sigmoid(xz_t + Uz h_{t-1}),  r_t = sigmoid(xr_t + Ur h_{t-1})

Sharding: data-parallel over batch across 8 NeuronCores (128 rows each);
weights replicated; no cross-core communication.

Design notes (the scan is latency-bound: ~2.2us/step serial chain):
- State lives transposed on-chip, [H=128 partitions, B_loc=128 free], so
  every matmul (weights stationary / state moving) needs no transposes.
  The host pre-transposes inputs to [T, I, B_loc], pre-broadcasts
  attention to [128, T*B_loc], and packs weights into lhsT layouts.
- Matmul operands and gate tensors are bf16 (fp32 PSUM accumulation,
  fp32 materialized h): fp32 matmuls run as HI/LO instruction pairs on
  trn2, so bf16 halves PE work and enables fast weight load.
- One PSUM bank per gate, 4 steps wide: matmul start=True zeroes the
  whole 2KB zero-region, so gates must not share a bank.  x-projections
  for 4 steps land in the bank up-front; recurrent parts accumulate.
- Deferred h-add: h_t = t2 - t1n (t1n = (a*z-1)*h, t2 = a*z*htilde) is
  never summed on the critical path; instead Ur/Uz are applied to t1n
  and t2 separately (negated weight copies compensate t1n's sign), so
  the next step's gate matmuls start one hop earlier.
- Critical cycle per step: tanh -> t2 -> Ur@t2 -> sigmoid_r -> r*h ->
  Uh@rh -> tanh (~6 engine hops; ACT/PE/DVE fixed costs dominate).
"""

import sys
import numpy as np
import ml_dtypes

for _p in ("/opt/trn_rl_repo",):
    if _p not in sys.path:
        sys.path.insert(0, _p)

import concourse.bacc as bacc
import concourse.mybir as mybir
import concourse.tile as tile
from concourse.bass_utils import run_bass_kernel_spmd

B, T, I, H = 1024, 200, 128, 128
NCORES = 8
BL = B // NCORES  # 128 batch rows per core
CH = 20           # t-steps per DMA chunk
F32 = mybir.dt.float32
BF16 = mybir.dt.bfloat16
AF = mybir.ActivationFunctionType
ALU = mybir.AluOpType
BF16NP = ml_dtypes.bfloat16

_compiled = None


def build_program():
    nc = bacc.Bacc("TRN2", target_bir_lowering=False, debug=False,
                   num_devices=NCORES)

    xT = nc.dram_tensor("xT", [T, I, BL], BF16, kind="ExternalInput").ap()
    aT = nc.dram_tensor("aT", [128, T * BL], BF16, kind="ExternalInput").ap()
    wcat_in = nc.dram_tensor("wcat", [128, 8 * 128], BF16,
                             kind="ExternalInput").ap()
    bcat_in = nc.dram_tensor("bcat", [128, 3], F32, kind="ExternalInput").ap()
    h_out = nc.dram_tensor("h_out", [H, BL], F32, kind="ExternalOutput").ap()

    with tile.TileContext(nc) as tc:
        with (
            tc.tile_pool(name="weights", bufs=1) as wpool,
            tc.tile_pool(name="xin", bufs=3) as xpool,
            tc.tile_pool(name="abc", bufs=2) as abcpool,
            tc.tile_pool(name="state", bufs=6) as spool,
            tc.tile_pool(name="ps", bufs=2, space="PSUM") as pspool,
        ):
            wcat = wpool.tile([128, 8 * 128], BF16, name="wcat", tag="wcat")
            nc.sync.dma_start(wcat[:], wcat_in)
            bcat = wpool.tile([128, 3], F32, name="bcat", tag="bcat")
            nc.sync.dma_start(bcat[:], bcat_in)
            wnames = ("wzx", "wrx", "whx", "uz", "ur", "uh", "uzn", "urn")
            w = {n: wcat[:, k * 128:(k + 1) * 128]
                 for k, n in enumerate(wnames)}
            w["bz"], w["br"], w["bh"] = (bcat[:, 0:1], bcat[:, 1:2],
                                         bcat[:, 2:3])

            warmup = spool.tile([128, 1], BF16, name="warmup", tag="warmup")
            nc.scalar.activation(warmup[:], bcat[:, 0:1], AF.Sigmoid)

            C = 4          # t-steps per batched x-projection / PSUM bank
            hparts = []    # tiles summing to h_{t-1} (deferred add)
            h = None       # materialized h_{t-1} (for hadamards)

            NG = T // C    # 50 groups of C=4 steps
            xg_t = {}      # group -> x tile [I, C*BL]
            ps_t = {}      # group -> (ps_z, ps_r, ps_h)
            abc = None

            def load_x(G):
                xg = xpool.tile([I, C * BL], BF16, name="xg", tag="xg",
                                bufs=6)[:]
                nc.sync.dma_start(
                    xg.rearrange("i (c b) -> i c b", c=C),
                    xT[G * C:(G + 1) * C].rearrange("c i b -> i c b"))
                xg_t[G] = xg

            def alloc_ps(G):
                ps_t[G] = (
                    pspool.tile([128, C * BL], F32, name="psz", tag="psz")[:],
                    pspool.tile([128, C * BL], F32, name="psr", tag="psr",
                                bufs=3)[:],
                    pspool.tile([128, C * BL], F32, name="psh", tag="psh",
                                bufs=3)[:])

            def emit_xproj(G, which, half=None):
                ps_z, ps_r, ps_h = ps_t[G]
                dst = {"r": ps_r, "z": ps_z, "h": ps_h}[which]
                wx = {"r": "wrx", "z": "wzx", "h": "whx"}[which]
                if half is None:
                    bi = nc.tensor.matmul(dst, w[wx], xg_t[G],
                                          start=True, stop=False)
                else:
                    cs = slice(half * 2 * BL, (half + 1) * 2 * BL)
                    bi = nc.tensor.matmul(dst[:, cs], w[wx], xg_t[G][:, cs],
                                          start=(half == 0), stop=False)
                return bi.ins

            load_x(0)
            load_x(1)
            alloc_ps(0)
            emit_xproj(0, "r")
            emit_xproj(0, "z")
            emit_xproj(0, "h", 0)
            emit_xproj(0, "h", 1)

            for G in range(NG):
                t0g = G * C
                # x DMA prefetched two groups ahead so it lands well before
                # the prefetch matmuls need it
                if G + 2 < NG:
                    load_x(G + 2)
                if t0g % CH == 0:
                    abc = abcpool.tile([128, CH * BL], BF16, name="abc",
                                       tag="abc")
                    nc.sync.dma_start(
                        abc[:], aT[:, t0g * BL:(t0g + CH) * BL])
                if G + 1 < NG:
                    alloc_ps(G + 1)
                ps_z, ps_r, ps_h = ps_t[G]

                for j in range(C):
                    t = t0g + j
                    sl = slice(j * BL, (j + 1) * BL)
                    av = abc[:, (t % CH) * BL:(t % CH + 1) * BL]
                    stop = j == C - 1
                    # recurrent parts: accumulate Ur/Uz applied to each
                    # addend of h_{t-1} (deferred h add)
                    for k, (hp, neg) in enumerate(hparts):
                        fin = stop and k == len(hparts) - 1
                        nc.tensor.matmul(ps_r[:, sl],
                                         w["urn" if neg else "ur"], hp[:],
                                         start=False, stop=fin)  # noqa
                    for k, (hp, neg) in enumerate(hparts):
                        fin = stop and k == len(hparts) - 1
                        nc.tensor.matmul(ps_z[:, sl],
                                         w["uzn" if neg else "uz"], hp[:],
                                         start=False, stop=fin)
                    # r-gate x-projection streams in the sigma_r window
                    # (different PSUM bank: no serialization with sigma_r)
                    if j == 1 and G + 1 < NG:
                        emit_xproj(G + 1, "r")
                    if h is not None:
                        r = spool.tile([H, BL], BF16, name="r", tag="r")
                        nc.scalar.activation(r[:], ps_r[:, sl], AF.Sigmoid,
                                             bias=w["br"])
                    z = spool.tile([H, BL], BF16, name="z", tag="z")
                    nc.scalar.activation(z[:], ps_z[:, sl], AF.Sigmoid,
                                         bias=w["bz"])
                    if h is not None:
                        rh = spool.tile([H, BL], BF16, name="rh", tag="rh")
                        nc.vector.tensor_mul(rh[:], r[:], h[:])
                        nc.tensor.matmul(ps_h[:, sl], w["uh"], rh[:],
                                         start=False, stop=stop)
                    # next group's x-projections stream in tanh windows;
                    # the h-gate is split so the group-boundary step stays
                    # clear of long matmuls
                    if j == 2 and G + 1 < NG:
                        emit_xproj(G + 1, "z")
                    ht = spool.tile([H, BL], BF16, name="ht", tag="ht")
                    nc.scalar.activation(ht[:], ps_h[:, sl], AF.Tanh,
                                         bias=w["bh"])
                    # emitted after tanh: same-bank pairs serialize in
                    # emission order, so tanh must come first
                    if j == 0 and G > 0:
                        emit_xproj(G, "h", 1)
                    elif j == 3 and G + 1 < NG:
                        emit_xproj(G + 1, "h", 0)
                    # z' = a*z ; h' = (1-z')*h + z'*ht  (add deferred)
                    zp = spool.tile([H, BL], BF16, name="zp", tag="zp")
                    nc.vector.tensor_mul(zp[:], z[:], av)
                    if h is not None:
                        # t1n = (zp-1)*h = -(1-zp)*h ; negated U weights
                        # compensate in the PSUM accumulates.  Emitted
                        # BEFORE t2 so it doesn't queue behind t2's
                        # tanh-wait in the DVE FIFO (its feeder matmul
                        # must clear the PE before t2 arrives).
                        t1n = spool.tile([H, BL], BF16, name="t1n", tag="t1n")
                        nc.vector.scalar_tensor_tensor(
                            t1n[:], zp[:], 1.0, h[:],
                            ALU.subtract, ALU.mult)
                    t2 = spool.tile([H, BL], BF16, name="t2", tag="t2")
                    nc.vector.tensor_mul(t2[:], zp[:], ht[:])
                    if h is None:
                        hparts = [(t2, False)]
                        h = t2
                    else:
                        hparts = [(t1n, True), (t2, False)]
                        if t == T - 1:
                            h = spool.tile([H, BL], F32, name="hf", tag="hf")
                        else:
                            h = spool.tile([H, BL], BF16, name="h", tag="h")
                        nc.vector.tensor_sub(h[:], t2[:], t1n[:])
                del xg_t[G], ps_t[G]

            nc.sync.dma_start(h_out, h[:])
    nc.compile()
    return nc


def _prep_inputs(inputs, attention_scores, Wz, bz, Wr, br, Wh, bh):
    """Shard + lay out host-side.  Returns per-core input maps."""
    x = np.asarray(inputs, dtype=np.float32)
    a = np.asarray(attention_scores, dtype=np.float32)
    Wz = np.asarray(Wz, dtype=np.float32)
    Wr = np.asarray(Wr, dtype=np.float32)
    Wh = np.asarray(Wh, dtype=np.float32)
    wcat = np.concatenate([
        Wz[:, :I].T, Wr[:, :I].T, Wh[:, :I].T,
        Wz[:, I:].T, Wr[:, I:].T, Wh[:, I:].T,
        -Wz[:, I:].T, -Wr[:, I:].T], axis=1)
    bcat = np.stack([np.asarray(bz, np.float32),
                     np.asarray(br, np.float32),
                     np.asarray(bh, np.float32)], axis=1)
    shared = {
        "wcat": np.ascontiguousarray(wcat).astype(BF16NP),
        "bcat": np.ascontiguousarray(bcat),
    }
    in_maps = []
    for c in range(NCORES):
        sl = slice(c * BL, (c + 1) * BL)
        in_maps.append({
            "xT": np.ascontiguousarray(
                x[sl].transpose(1, 2, 0)).astype(BF16NP),
            "aT": np.ascontiguousarray(np.broadcast_to(
                a[sl].T.reshape(1, T * BL), (128, T * BL))).astype(BF16NP),
            **shared,
        })
    return in_maps


def kernel(inputs, attention_scores, Wz, bz, Wr, br, Wh, bh):
    global _compiled
    if _compiled is None:
        _compiled = build_program()
    nc = _compiled
    in_maps = _prep_inputs(inputs, attention_scores, Wz, bz, Wr, br, Wh, bh)
    res = run_bass_kernel_spmd(nc, in_maps, list(range(NCORES)))
    out = np.empty((B, H), dtype=np.float32)
    for c in range(NCORES):
        out[c * BL:(c + 1) * BL, :] = res.results[c]["h_out"].T
    return out



# revision 2
# speedup vs baseline: 1.1144x; 1.1144x over previous
"""AUGRU Trainium2 kernel — 2-chain pipelined, parameterized emission order.

See kernel_v2.py docstring for the design rationale.  This variant factors
the per-(step, chain) work into stage closures and supports several
emission templates (EMIT_MODE) to control each in-order engine queue:

  1: stage-interleaved   [mmA mmB | sA sB | dA dB | uA uB | tA tB | eA eB]
  2: body-interleaved    [A: mm s d u t e][B: mm s d u t e]
  3: half-step shifted   per t: [A-half1(t), B-half2(t-1), A-half2(t),
                                 B-half1(t)]
"""

import sys
import numpy as np
import ml_dtypes

for _p in ("/opt/trn_rl_repo",):
    if _p not in sys.path:
        sys.path.insert(0, _p)

import concourse.bacc as bacc
import concourse.mybir as mybir
import concourse.tile as tile
from concourse.bass_utils import run_bass_kernel_spmd

B, T, I, H = 1024, 200, 128, 128
NCORES = 8
BL = B // NCORES   # 128 batch rows per core
BC = BL // 2       # 64 batch rows per chain
C = 4              # t-steps per x-projection group
NG = T // C        # 50 groups
CH = 20            # t-steps per attention DMA chunk
F32 = mybir.dt.float32
BF16 = mybir.dt.bfloat16
AF = mybir.ActivationFunctionType
ALU = mybir.AluOpType
BF16NP = ml_dtypes.bfloat16

EMIT_MODE = 3

_compiled = None


def build_program():
    nc = bacc.Bacc("TRN2", target_bir_lowering=False, debug=False,
                   num_devices=NCORES)

    xT = nc.dram_tensor("xT", [T, I, BL], BF16, kind="ExternalInput").ap()
    aT = nc.dram_tensor("aT", [128, T * BL], BF16, kind="ExternalInput").ap()
    wcat_in = nc.dram_tensor("wcat", [128, 8 * 128], BF16,
                             kind="ExternalInput").ap()
    brow_in = nc.dram_tensor("brow", [1, 2 * 128], BF16,
                             kind="ExternalInput").ap()
    ones_in = nc.dram_tensor("ones", [1, C * BC], BF16,
                             kind="ExternalInput").ap()
    bh_in = nc.dram_tensor("bh", [128, 1], F32, kind="ExternalInput").ap()
    h_out = nc.dram_tensor("h_out", [H, BL], F32, kind="ExternalOutput").ap()

    with tile.TileContext(nc) as tc:
        with (
            tc.tile_pool(name="weights", bufs=1) as wpool,
            tc.tile_pool(name="xin", bufs=5) as xpool,
            tc.tile_pool(name="abc", bufs=2) as abcpool,
            tc.tile_pool(name="state", bufs=4) as spool,
            tc.tile_pool(name="ps", bufs=2, space="PSUM") as pspool,
        ):
            wcat = wpool.tile([128, 8 * 128], BF16, name="wcat", tag="wcat")
            nc.sync.dma_start(wcat[:], wcat_in)
            brow = wpool.tile([1, 2 * 128], BF16, name="brow", tag="brow")
            nc.sync.dma_start(brow[:], brow_in)
            ones = wpool.tile([1, C * BC], BF16, name="ones", tag="ones")
            nc.sync.dma_start(ones[:], ones_in)
            bh = wpool.tile([128, 1], F32, name="bh", tag="bh")
            nc.sync.dma_start(bh[:], bh_in)
            wnames = ("wrx", "wzx", "whx", "ur", "uz", "uh", "urn", "uzn")
            w = {n: wcat[:, k * 128:(k + 1) * 128]
                 for k, n in enumerate(wnames)}

            warmup = spool.tile([128, 1], BF16, name="warmup", tag="warmup",
                                bufs=1)
            nc.scalar.activation(warmup[:], bh[:], AF.Sigmoid)

            GW = 2 * C * BC      # 512 psum cols per [r|z] tile
            h = [None, None]     # per-chain materialized h_{t-1}
            hparts = [[], []]    # per-chain [(tile, negated)] summing to h
            xg_t = {}
            psrz_t = {}
            psh_t = {}
            abc = {}             # chunk index -> attention tile
            rz_t, zp_t, t1n_t, rh_t, ht_t = {}, {}, {}, {}, {}

            def load_x(g):
                for X in (0, 1):
                    xg = xpool.tile([I, C * BC], BF16, name=f"xg{X}",
                                    tag=f"xg{X}")[:]
                    nc.sync.dma_start(
                        xg.rearrange("i (c b) -> i c b", c=C),
                        xT[g * C:(g + 1) * C, :,
                           X * BC:(X + 1) * BC].rearrange("c i b -> i c b"))
                    xg_t[(g, X)] = xg

            def load_abc(k):
                a = abcpool.tile([128, CH * BL], BF16, name="abc",
                                 tag="abc")[:]
                nc.sync.dma_start(a, aT[:, k * CH * BL:(k + 1) * CH * BL])
                abc[k] = a

            def alloc_ps(g):
                # one full PSUM bank per tile: [r|z] per chain, h per chain
                # (separate h tiles per chain so the tile-granularity
                # write-after-read tracking never couples the two chains'
                # Uh-matmul/tanh cycles; 4 tags x bufs=2 = all 8 banks)
                psrz_t[(g, 0)] = pspool.tile([128, GW], F32, name="psrzA",
                                             tag="psrzA")[:]
                psrz_t[(g, 1)] = pspool.tile([128, GW], F32, name="psrzB",
                                             tag="psrzB")[:]
                psh_t[(g, 0)] = pspool.tile([128, GW], F32, name="pshA",
                                            tag="pshA")[:]
                psh_t[(g, 1)] = pspool.tile([128, GW], F32, name="pshB",
                                            tag="pshB")[:]

            # all boundary matmuls are emitted in 128-col pieces: the tile
            # scheduler places them from its (inaccurate) cost model, and a
            # misplaced 600ns matmul head-of-line-blocks the critical
            # recurrent matmuls, while a 128-col piece costs <=150ns.
            HB = C * BC // 2    # 128 cols = half a gate's group block

            def emit_xproj_rz(g, X, piece):
                ps = psrz_t[(g, X)]
                lo, hi = piece * HB, (piece + 1) * HB
                nc.tensor.matmul(ps[:, lo:hi], w["wrx"],
                                 xg_t[(g, X)][:, lo:hi],
                                 start=(piece == 0), stop=False)
                nc.tensor.matmul(ps[:, C * BC + lo:C * BC + hi], w["wzx"],
                                 xg_t[(g, X)][:, lo:hi],
                                 start=False, stop=False)

            def emit_bias_rz(g, X, piece):
                ps = psrz_t[(g, X)]
                lo, hi = piece * HB, (piece + 1) * HB
                nc.tensor.matmul(ps[:, lo:hi], brow[:, 0:128],
                                 ones[:, 0:HB], start=False, stop=False)
                nc.tensor.matmul(ps[:, C * BC + lo:C * BC + hi],
                                 brow[:, 128:256], ones[:, 0:HB],
                                 start=False, stop=False)

            def emit_xproj_h(g, X, piece):
                lo, hi = piece * HB, (piece + 1) * HB
                nc.tensor.matmul(psh_t[(g, X)][:, lo:hi],
                                 w["whx"], xg_t[(g, X)][:, lo:hi],
                                 start=(piece == 0), stop=False)

            # ---- per-(step, chain) stages -------------------------------
            def st_mms(t, X):
                g, j = divmod(t, C)
                ps = psrz_t[(g, X)]
                rsl = slice(j * BC, (j + 1) * BC)
                zsl = slice(C * BC + j * BC, C * BC + (j + 1) * BC)
                n = len(hparts[X])
                for k, (hp, neg) in enumerate(hparts[X]):
                    nc.tensor.matmul(ps[:, rsl],
                                     w["urn" if neg else "ur"], hp[:],
                                     start=False, stop=False)
                    nc.tensor.matmul(ps[:, zsl],
                                     w["uzn" if neg else "uz"], hp[:],
                                     start=False, stop=(k == n - 1))

            def st_sigma(t, X):
                g, j = divmod(t, C)
                rz = spool.tile([H, 2 * BC], BF16, name=f"rz{X}",
                                tag=f"rz{X}")
                ps_view = psrz_t[(g, X)].rearrange(
                    "p (g c) -> p g c", g=2)[:, :, j * BC:(j + 1) * BC]
                nc.scalar.activation(
                    rz[:].rearrange("p (g c) -> p g c", g=2),
                    ps_view, AF.Sigmoid)
                rz_t[(t, X)] = rz

            def st_dve1(t, X):
                rz = rz_t[(t, X)]
                if h[X] is not None:
                    rh = spool.tile([H, BC], BF16, name=f"rh{X}",
                                    tag=f"rh{X}")
                    nc.vector.tensor_mul(rh[:], rz[:, 0:BC], h[X][:])
                    rh_t[(t, X)] = rh
                av = abc[t // CH][:, (t % CH) * BL + X * BC:
                                 (t % CH) * BL + (X + 1) * BC]
                zp = spool.tile([H, BC], BF16, name=f"zp{X}", tag=f"zp{X}")
                nc.vector.tensor_mul(zp[:], rz[:, BC:2 * BC], av)
                zp_t[(t, X)] = zp
                if h[X] is not None:
                    t1n = spool.tile([H, BC], BF16, name=f"t1n{X}",
                                     tag=f"t1n{X}")
                    nc.vector.scalar_tensor_tensor(
                        t1n[:], zp[:], 1.0, h[X][:],
                        ALU.subtract, ALU.mult)
                    t1n_t[(t, X)] = t1n

            def st_uh(t, X):
                g, j = divmod(t, C)
                if (t, X) in rh_t:
                    hsl = slice(j * BC, (j + 1) * BC)
                    nc.tensor.matmul(psh_t[(g, X)][:, hsl], w["uh"],
                                     rh_t[(t, X)][:], start=False, stop=True)

            def st_tanh(t, X):
                g, j = divmod(t, C)
                hsl = slice(j * BC, (j + 1) * BC)
                ht = spool.tile([H, BC], BF16, name=f"ht{X}", tag=f"ht{X}")
                nc.scalar.activation(ht[:], psh_t[(g, X)][:, hsl], AF.Tanh,
                                     bias=bh[:])
                ht_t[(t, X)] = ht

            def st_dve2(t, X):
                t2 = spool.tile([H, BC], BF16, name=f"t2{X}", tag=f"t2{X}")
                nc.vector.tensor_mul(t2[:], zp_t[(t, X)][:],
                                     ht_t[(t, X)][:])
                if h[X] is None:
                    hparts[X] = [(t2, False)]
                    h[X] = t2
                else:
                    t1n = t1n_t[(t, X)]
                    hparts[X] = [(t1n, True), (t2, False)]
                    if t == T - 1:
                        hn = spool.tile([H, BC], F32, name=f"hf{X}",
                                        tag=f"hf{X}", bufs=1)
                    else:
                        hn = spool.tile([H, BC], BF16, name=f"h{X}",
                                        tag=f"h{X}")
                    nc.vector.tensor_sub(hn[:], t2[:], t1n[:])
                    h[X] = hn

            def st_groupwork(t):
                g, j = divmod(t, C)
                if j == 0 and g + 2 < NG:
                    load_x(g + 2)
                if t % CH == 0 and t + CH < T:
                    load_abc(t // CH + 1)
                if j == 0 and g + 1 < NG:
                    alloc_ps(g + 1)

            # all of group g+1's x-proj/bias matmuls become data-ready at
            # the start of group g (x prefetched 2 groups out, PSUM bank
            # released), so the greedy scheduler front-loads all ~10 of
            # them into one PE blob that head-of-line-blocks the critical
            # recurrent matmuls.  bass_wait_until_ts (a scheduling-pass
            # hold, no HW effect) spreads them across the group's steps.
            P_EST_NS = 1550.0

            def st_grouppe(t):
                g, j = divmod(t, C)
                if g + 1 >= NG:
                    return

                def hold(t_eff):
                    return tc.tile_wait_until(t_eff * P_EST_NS * 1e-6)

                if j == 1:
                    for k, (X, piece) in enumerate(
                            ((0, 0), (0, 1), (1, 0), (1, 1))):
                        with hold(g * C + 1 + 0.25 * k):
                            emit_xproj_rz(g + 1, X, piece)
                elif j == 2:
                    for k, (X, piece) in enumerate(
                            ((0, 0), (0, 1), (1, 0), (1, 1))):
                        with hold(g * C + 2 + 0.25 * k):
                            emit_xproj_h(g + 1, X, piece)
                elif j == 3:
                    for k, (X, piece) in enumerate(
                            ((0, 0), (0, 1), (1, 0), (1, 1))):
                        with hold(g * C + 3 + 0.25 * k):
                            emit_bias_rz(g + 1, X, piece)

            # ---- emission templates -------------------------------------
            load_x(0)
            load_x(1)
            load_abc(0)
            alloc_ps(0)
            for X in (0, 1):
                for piece in (0, 1):
                    emit_xproj_rz(0, X, piece)
                    emit_bias_rz(0, X, piece)
                    emit_xproj_h(0, X, piece)

            def half1(t, X):
                st_mms(t, X)
                st_sigma(t, X)
                st_dve1(t, X)

            def half2(t, X):
                st_uh(t, X)
                st_tanh(t, X)
                st_dve2(t, X)

            if EMIT_MODE == 1:
                for t in range(T):
                    st_groupwork(t)
                    st_mms(t, 0)
                    st_mms(t, 1)
                    st_grouppe(t)
                    st_sigma(t, 0)
                    st_sigma(t, 1)
                    st_dve1(t, 0)
                    st_dve1(t, 1)
                    st_uh(t, 0)
                    st_uh(t, 1)
                    st_tanh(t, 0)
                    st_tanh(t, 1)
                    st_dve2(t, 0)
                    st_dve2(t, 1)
            elif EMIT_MODE == 2:
                for t in range(T):
                    st_groupwork(t)
                    half1(t, 0)
                    st_grouppe(t)
                    half2(t, 0)
                    half1(t, 1)
                    half2(t, 1)
            elif EMIT_MODE == 3:
                for t in range(T):
                    st_groupwork(t)
                    half1(t, 0)
                    st_grouppe(t)
                    if t > 0:
                        half2(t - 1, 1)
                    half2(t, 0)
                    half1(t, 1)
                half2(T - 1, 1)
            else:
                raise ValueError(EMIT_MODE)

            nc.sync.dma_start(h_out[:, 0:BC], h[0][:])
            nc.sync.dma_start(h_out[:, BC:BL], h[1][:])
    nc.compile()
    return nc


def _prep_inputs(inputs, attention_scores, Wz, bz, Wr, br, Wh, bh):
    """Shard + lay out host-side.  Returns per-core input maps."""
    x = np.asarray(inputs, dtype=np.float32)
    a = np.asarray(attention_scores, dtype=np.float32)
    Wz = np.asarray(Wz, dtype=np.float32)
    Wr = np.asarray(Wr, dtype=np.float32)
    Wh = np.asarray(Wh, dtype=np.float32)
    wcat = np.concatenate([
        Wr[:, :I].T, Wz[:, :I].T, Wh[:, :I].T,
        Wr[:, I:].T, Wz[:, I:].T, Wh[:, I:].T,
        -Wr[:, I:].T, -Wz[:, I:].T], axis=1)
    brow = np.concatenate([np.asarray(br, np.float32),
                           np.asarray(bz, np.float32)])[None, :]
    shared = {
        "wcat": np.ascontiguousarray(wcat).astype(BF16NP),
        "brow": np.ascontiguousarray(brow).astype(BF16NP),
        "ones": np.ones((1, C * BC), dtype=BF16NP),
        "bh": np.ascontiguousarray(
            np.asarray(bh, np.float32).reshape(128, 1)),
    }
    in_maps = []
    for c in range(NCORES):
        sl = slice(c * BL, (c + 1) * BL)
        in_maps.append({
            "xT": np.ascontiguousarray(
                x[sl].transpose(1, 2, 0)).astype(BF16NP),
            "aT": np.ascontiguousarray(np.broadcast_to(
                a[sl].T.reshape(1, T * BL), (128, T * BL))).astype(BF16NP),
            **shared,
        })
    return in_maps


def kernel(inputs, attention_scores, Wz, bz, Wr, br, Wh, bh):
    global _compiled
    if _compiled is None:
        _compiled = build_program()
    nc = _compiled
    in_maps = _prep_inputs(inputs, attention_scores, Wz, bz, Wr, br, Wh, bh)
    res = run_bass_kernel_spmd(nc, in_maps, list(range(NCORES)))
    out = np.empty((B, H), dtype=np.float32)
    for c in range(NCORES):
        out[c * BL:(c + 1) * BL, :] = res.results[c]["h_out"].T
    return out
